# revision 4
# baseline (speedup 1.0000x reference)
"""Trainium2 Bass kernel for nn_Jastrow (1024-electron pairwise Jastrow factor).

Strategy (8 NeuronCores, data-parallel over pair rows):
  - Core k owns electron rows i in [128k, 128k+128) and ALL j: 128x1024 pairs.
  - Pair features are generated on-chip in [128 i, 1024 j] layout from a
    host-broadcast copy of the electron coordinates (no 1M-row gather ever
    touches HBM).  Columns are host-permuted so j in [0,512) is always the
    same-spin half for that core's rows.
  - The two tiny pair MLPs (4->64->64, tanh) run on the PE in float32r with
    TWO pairs packed per moving column (block-diagonal weights, K=8 / K=128,
    M=128), so PE and ACT process 2 pairs per column.
  - The 64->1 output layer and all scalar postprocessing (softplus, sqrt,
    log, sign, Yukawa prefactors, diagonal-pair correction) collapse into a
    host-side fp64 epilogue on the 8 cores' partial sums:
        sum_p mlp(f_p) = W2 . sum_p h2_p
  - Yukawa cusp term expm1(-d/F)/d is computed per pair in fp32 on DVE/ACT
    and free-dim-reduced on the fly (fused (e-1)*u with accum_out).
  - Per-core device output is just [128, 8] of partial sums.
"""
import os
import sys

sys.path.insert(0, "/opt/trn_rl_repo")

import numpy as np

import concourse.bacc as bacc
import concourse.mybir as mybir
from concourse import tile
from concourse.bass_utils import run_bass_kernel_spmd

AF = mybir.ActivationFunctionType
OP = mybir.AluOpType
F32 = mybir.dt.float32
F32R = mybir.dt.float32r

N_EL = 1024
N_UP = 512
D_EMB = 256
WIDTH = 64
NC = 8
ROWS = N_EL // NC  # 128 i-rows per core
HALF = 512  # j-columns per spin half
PACK_COLS = ROWS * HALF // 2  # 32768 packed pair-columns per half (2 pairs/col)
CHUNK = 8192  # packed columns per feats tile
PSCH = 2048  # packed columns per PSUM tile (4 banks)


def _build_program(exp_scale_same, exp_scale_diff):
    nc = bacc.Bacc("TRN2", target_bir_lowering=False, debug=False)

    # ---- I/O ----
    elb_in = nc.dram_tensor("elb", [3, ROWS, N_EL], F32, kind="ExternalInput")
    eli_in = nc.dram_tensor("eli", [ROWS, 3], F32, kind="ExternalInput")
    nm_in = nc.dram_tensor("nm", [ROWS, N_EL], F32, kind="ExternalInput")
    w0s_in = nc.dram_tensor("w0s", [8, 128], F32R, kind="ExternalInput")
    w1s_in = nc.dram_tensor("w1s", [128, 128], F32R, kind="ExternalInput")
    w0d_in = nc.dram_tensor("w0d", [8, 128], F32R, kind="ExternalInput")
    w1d_in = nc.dram_tensor("w1d", [128, 128], F32R, kind="ExternalInput")
    b0s_in = nc.dram_tensor("b0s", [128, 1], F32, kind="ExternalInput")
    b1s_in = nc.dram_tensor("b1s", [128, 1], F32, kind="ExternalInput")
    b0d_in = nc.dram_tensor("b0d", [128, 1], F32, kind="ExternalInput")
    b1d_in = nc.dram_tensor("b1d", [128, 1], F32, kind="ExternalInput")
    embt_in = nc.dram_tensor("embt", [2, 128, ROWS], F32, kind="ExternalInput")
    we0_in = nc.dram_tensor("we0", [2, 128, WIDTH], F32, kind="ExternalInput")
    we1_in = nc.dram_tensor("we1", [WIDTH, WIDTH], F32, kind="ExternalInput")
    be0_in = nc.dram_tensor("be0", [WIDTH, 1], F32, kind="ExternalInput")
    be1_in = nc.dram_tensor("be1", [WIDTH, 1], F32, kind="ExternalInput")
    out_dram = nc.dram_tensor("out", [128, 8], F32, kind="ExternalOutput")

    with tile.TileContext(nc) as tc:
        with (
            tc.tile_pool(name="cst", bufs=1) as cst,
            tc.tile_pool(name="wrk", bufs=2) as wrk,
            tc.tile_pool(name="psum", bufs=1, space="PSUM") as psum,
        ):
            # ---- load persistent tiles ----
            elbx = cst.tile([ROWS, N_EL], F32, tag="elbx")
            elby = cst.tile([ROWS, N_EL], F32, tag="elby")
            elbz = cst.tile([ROWS, N_EL], F32, tag="elbz")
            nc.sync.dma_start(elbx[:], elb_in[0])
            nc.sync.dma_start(elby[:], elb_in[1])
            nc.sync.dma_start(elbz[:], elb_in[2])
            eli = cst.tile([ROWS, 3], F32, tag="eli")
            nc.sync.dma_start(eli[:], eli_in[:])
            nm = cst.tile([ROWS, N_EL], F32, tag="nm")
            nc.sync.dma_start(nm[:], nm_in[:])

            w0 = {}
            w1 = {}
            b0 = {}
            b1 = {}
            for h, (w0_in_, w1_in_, b0_in_, b1_in_) in enumerate(
                [(w0s_in, w1s_in, b0s_in, b1s_in), (w0d_in, w1d_in, b0d_in, b1d_in)]
            ):
                w0[h] = cst.tile([8, 128], F32R, tag=f"w0_{h}", name=f"w0_{h}")
                nc.sync.dma_start(w0[h][:], w0_in_[:])
                w1[h] = cst.tile([128, 128], F32R, tag=f"w1_{h}", name=f"w1_{h}")
                nc.sync.dma_start(w1[h][:], w1_in_[:])
                b0[h] = cst.tile([128, 1], F32, tag=f"b0_{h}", name=f"b0_{h}")
                nc.sync.dma_start(b0[h][:], b0_in_[:])
                b1[h] = cst.tile([128, 1], F32, tag=f"b1_{h}", name=f"b1_{h}")
                nc.sync.dma_start(b1[h][:], b1_in_[:])

            embt = cst.tile([128, 2, ROWS], F32, tag="embt")
            nc.sync.dma_start(embt[:, 0, :], embt_in[0])
            nc.sync.dma_start(embt[:, 1, :], embt_in[1])
            we0 = cst.tile([128, 2, WIDTH], F32, tag="we0")
            nc.sync.dma_start(we0[:, 0, :], we0_in[0])
            nc.sync.dma_start(we0[:, 1, :], we0_in[1])
            we1 = cst.tile([WIDTH, WIDTH], F32, tag="we1")
            nc.sync.dma_start(we1[:], we1_in[:])
            be0 = cst.tile([WIDTH, 1], F32, tag="be0")
            nc.sync.dma_start(be0[:], be0_in[:])
            be1 = cst.tile([WIDTH, 1], F32, tag="be1")
            nc.sync.dma_start(be1[:], be1_in[:])

            # ---- pair features, [128 i, 1024 j] planes ----
            # d{x,y,z} = el_i - el_j  (one fused op each)
            dx = cst.tile([ROWS, N_EL], F32, tag="dx")
            dy = cst.tile([ROWS, N_EL], F32, tag="dy")
            dz = cst.tile([ROWS, N_EL], F32, tag="dz")
            nc.vector.tensor_scalar(dx[:], elbx[:], -1.0, eli[:, 0:1], OP.mult, OP.add)
            nc.vector.tensor_scalar(dy[:], elby[:], -1.0, eli[:, 1:2], OP.mult, OP.add)
            nc.vector.tensor_scalar(dz[:], elbz[:], -1.0, eli[:, 2:3], OP.mult, OP.add)

            sq0 = cst.tile([ROWS, N_EL], F32, tag="sq0")
            sq1 = cst.tile([ROWS, N_EL], F32, tag="sq1")
            r2 = cst.tile([ROWS, N_EL], F32, tag="r2")
            nc.vector.tensor_tensor(sq0[:], dx[:], dx[:], OP.mult)
            nc.vector.tensor_tensor(sq1[:], dy[:], dy[:], OP.mult)
            nc.vector.tensor_tensor(sq0[:], sq0[:], sq1[:], OP.add)
            nc.vector.tensor_tensor(sq1[:], dz[:], dz[:], OP.mult)
            nc.vector.tensor_tensor(r2[:], sq0[:], sq1[:], OP.add)

            s = cst.tile([ROWS, N_EL], F32, tag="s")  # r = |diff|
            nc.scalar.activation(s[:], r2[:], AF.Sqrt)
            rs = cst.tile([ROWS, N_EL], F32, tag="rs")  # r + [i==j]
            nc.vector.tensor_tensor(rs[:], s[:], nm[:], OP.add)
            u = cst.tile([ROWS, N_EL], F32, tag="u")  # 1/rs
            nc.vector.reciprocal(u[:], rs[:])
            t = cst.tile([ROWS, N_EL], F32, tag="t")  # log1p(r)
            nc.scalar.activation(t[:], s[:], AF.Ln, bias=1.0)
            w = cst.tile([ROWS, N_EL], F32, tag="w")  # log1p(r)/rs
            nc.vector.tensor_tensor(w[:], t[:], u[:], OP.mult)

            # f32r feature planes for the MLP
            dxw = cst.tile([ROWS, N_EL], F32R, tag="dxw")
            dyw = cst.tile([ROWS, N_EL], F32R, tag="dyw")
            dzw = cst.tile([ROWS, N_EL], F32R, tag="dzw")
            tfr = cst.tile([ROWS, N_EL], F32R, tag="tfr")
            nc.vector.tensor_tensor(dxw[:], dx[:], w[:], OP.mult)
            nc.vector.tensor_tensor(dyw[:], dy[:], w[:], OP.mult)
            nc.vector.tensor_tensor(dzw[:], dz[:], w[:], OP.mult)
            nc.vector.tensor_copy(tfr[:], t[:])

            # ---- Yukawa cusp: sum over pairs of expm1(-r/F)/r, per half ----
            yukred = {}
            for h, esc in enumerate([exp_scale_same, exp_scale_diff]):
                cols = slice(h * HALF, (h + 1) * HALF)
                e = wrk.tile([ROWS, HALF], F32, tag="e")
                nc.scalar.activation(e[:], s[:, cols], AF.Exp, scale=float(esc))
                ydump = wrk.tile([ROWS, HALF], F32, tag="ydump")
                yukred[h] = cst.tile([ROWS, 1], F32, tag=f"yukred{h}", name=f"yukred{h}")
                nc.vector.scalar_tensor_tensor(
                    ydump[:], e[:], 1.0, u[:, cols], OP.subtract, OP.mult,
                    accum_out=yukred[h][:],
                )

            # ---- pair MLPs: pack 2 pairs per column, f32r matmuls ----
            planes = (dxw, dyw, dzw, tfr)
            accred = {}
            for h in (0, 1):
                acc = cst.tile([128, PACK_COLS // PSCH], F32, tag=f"acc{h}")
                for c in range(PACK_COLS // CHUNK):
                    f8 = wrk.tile([8, CHUNK], F32R, tag="f8")
                    rows_per = CHUNK // HALF  # 16 i-rows per chunk per group
                    for g in (0, 1):
                        r0 = 64 * g + rows_per * c
                        for pl, plane in enumerate(planes):
                            p = 4 * g + pl
                            nc.sync.dma_start(
                                f8[p : p + 1, :],
                                plane[r0 : r0 + rows_per, h * HALF : (h + 1) * HALF],
                            )
                    for q in range(CHUNK // PSCH):
                        ps_a = psum.tile([128, PSCH], F32, tag="A")
                        for r in range(PSCH // 512):
                            c0 = PSCH * q + 512 * r
                            nc.tensor.matmul(
                                ps_a[:, 512 * r : 512 * (r + 1)],
                                w0[h][:],
                                f8[:, c0 : c0 + 512],
                                start=True,
                                stop=True,
                            )
                        h1 = wrk.tile([128, PSCH], F32R, tag="h1")
                        nc.scalar.activation(h1[:], ps_a[:], AF.Tanh, bias=b0[h][:])
                        ps_b = psum.tile([128, PSCH], F32, tag="B")
                        for r in range(PSCH // 512):
                            nc.tensor.matmul(
                                ps_b[:, 512 * r : 512 * (r + 1)],
                                w1[h][:],
                                h1[:, 512 * r : 512 * (r + 1)],
                                start=True,
                                stop=True,
                            )
                        scrap = wrk.tile([128, PSCH], F32, tag="scrap")
                        idx = c * (CHUNK // PSCH) + q
                        nc.scalar.activation(
                            scrap[:], ps_b[:], AF.Tanh, bias=b1[h][:],
                            accum_out=acc[:, idx : idx + 1],
                        )
                accred[h] = cst.tile([128, 1], F32, tag=f"accred{h}", name=f"accred{h}")
                nc.vector.tensor_reduce(accred[h][:], acc[:], mybir.AxisListType.X, OP.add)

            # ---- per-electron embedding MLP (rows i0..i0+127 of embeddings) ----
            ps_e = psum.tile([WIDTH, ROWS], F32, tag="A")
            nc.tensor.matmul(ps_e[:], we0[:, 0, :], embt[:, 0, :], start=True, stop=False)
            nc.tensor.matmul(ps_e[:], we0[:, 1, :], embt[:, 1, :], start=False, stop=True)
            h1e = cst.tile([WIDTH, ROWS], F32, tag="h1e")
            nc.scalar.activation(h1e[:], ps_e[:], AF.Tanh, bias=be0[:])
            ps_e2 = psum.tile([WIDTH, ROWS], F32, tag="B")
            nc.tensor.matmul(ps_e2[:], we1[:], h1e[:], start=True, stop=True)
            h2e = cst.tile([WIDTH, ROWS], F32, tag="h2e")
            h2eacc = cst.tile([WIDTH, 1], F32, tag="h2eacc")
            nc.scalar.activation(
                h2e[:], ps_e2[:], AF.Tanh, bias=be1[:], accum_out=h2eacc[:]
            )

            # ---- outputs ----
            nc.sync.dma_start(out_dram[:, 0:1], yukred[0][:])
            nc.sync.dma_start(out_dram[:, 1:2], yukred[1][:])
            nc.sync.dma_start(out_dram[:, 2:3], accred[0][:])
            nc.sync.dma_start(out_dram[:, 3:4], accred[1][:])
            nc.sync.dma_start(out_dram[0:WIDTH, 4:5], h2eacc[:])

    nc.compile()
    return nc


_CACHE = {}


def _softplus(x):
    x = np.float64(x)
    return np.logaddexp(0.0, x)


def kernel(
    electrons, embeddings, A_same, A_diff,
    Ws0_same, bs0_same, Ws1_same, bs1_same, Ws2_same,
    Ws0_diff, bs0_diff, Ws1_diff, bs1_diff, Ws2_diff,
    scale_same, scale_diff,
    We0, be0, We1, be1, We2, be2, mlp_scale, log_bias,
):
    el = np.asarray(electrons, np.float32)
    emb = np.asarray(embeddings, np.float32)
    A_s64 = float(np.asarray(A_same, np.float64))
    A_d64 = float(np.asarray(A_diff, np.float64))
    W0s = np.asarray(Ws0_same, np.float32)
    W1s = np.asarray(Ws1_same, np.float32)
    W2s = np.asarray(Ws2_same, np.float32)
    b0s = np.asarray(bs0_same, np.float32)
    b1s = np.asarray(bs1_same, np.float32)
    W0d = np.asarray(Ws0_diff, np.float32)
    W1d = np.asarray(Ws1_diff, np.float32)
    W2d = np.asarray(Ws2_diff, np.float32)
    b0d = np.asarray(bs0_diff, np.float32)
    b1d = np.asarray(bs1_diff, np.float32)
    We0_ = np.asarray(We0, np.float32)
    We1_ = np.asarray(We1, np.float32)
    We2_ = np.asarray(We2, np.float32)
    be0_ = np.asarray(be0, np.float32)
    be1_ = np.asarray(be1, np.float32)
    be2_ = np.asarray(be2, np.float32)
    mscale = np.asarray(mlp_scale, np.float64)
    lbias = float(np.asarray(log_bias, np.float64))
    sc_s = float(np.asarray(scale_same, np.float64))
    sc_d = float(np.asarray(scale_diff, np.float64))

    A_sp_s = _softplus(A_s64)
    A_sp_d = _softplus(A_d64)
    F_s = np.sqrt(2.0 * A_sp_s)
    F_d = np.sqrt(2.0 * A_sp_d)

    key = (round(-1.0 / F_s, 12), round(-1.0 / F_d, 12))
    if key not in _CACHE:
        _CACHE[key] = _build_program(-1.0 / F_s, -1.0 / F_d)
    nc = _CACHE[key]

    # ---- block-diagonal packed weights (2 pair-groups per column) ----
    def blk(W0_, W1_, b0_, b1_):
        w0b = np.zeros((8, 128), np.float32)
        w0b[0:4, 0:64] = W0_
        w0b[4:8, 64:128] = W0_
        w1b = np.zeros((128, 128), np.float32)
        w1b[0:64, 0:64] = W1_
        w1b[64:128, 64:128] = W1_
        b0b = np.concatenate([b0_, b0_]).reshape(128, 1)
        b1b = np.concatenate([b1_, b1_]).reshape(128, 1)
        return w0b, w1b, b0b, b1b

    w0bs, w1bs, b0bs, b1bs = blk(W0s, W1s, b0s, b1s)
    w0bd, w1bd, b0bd, b1bd = blk(W0d, W1d, b0d, b1d)

    embT = emb.T.copy()  # [256, 1024]
    we0v = np.ascontiguousarray(We0_.reshape(2, 128, WIDTH))

    in_maps = []
    for k in range(NC):
        i0 = ROWS * k
        if i0 < N_UP:
            perm = np.arange(N_EL)
        else:
            perm = np.concatenate([np.arange(N_UP, N_EL), np.arange(0, N_UP)])
        elp = el[perm]  # [1024, 3] permuted so own-spin js come first
        elb = np.ascontiguousarray(
            np.broadcast_to(elp.T[:, None, :], (3, ROWS, N_EL)), np.float32
        )
        nmv = np.zeros((ROWS, N_EL), np.float32)
        rows = np.arange(ROWS)
        # global j == i0+p sits at permuted position (i0+p) % 512 in the
        # own-spin half (always columns [0, 512))
        nmv[rows, (i0 + rows) % N_UP] = 1.0
        in_maps.append(
            dict(
                elb=elb,
                eli=np.ascontiguousarray(el[i0 : i0 + ROWS]),
                nm=nmv,
                w0s=w0bs, w1s=w1bs, w0d=w0bd, w1d=w1bd,
                b0s=b0bs, b1s=b1bs, b0d=b0bd, b1d=b1bd,
                embt=np.ascontiguousarray(
                    embT[:, i0 : i0 + ROWS].reshape(2, 128, ROWS)
                ),
                we0=we0v,
                we1=We1_,
                be0=be0_.reshape(WIDTH, 1),
                be1=be1_.reshape(WIDTH, 1),
            )
        )

    trace = bool(int(os.environ.get("KERNEL_TRACE", "0")))
    res = run_bass_kernel_spmd(nc, in_maps, list(range(NC)), trace=trace)
    if trace:
        print(f"HW exec time: {res.exec_time_ns} ns")
        kernel.last_exec_time_ns = res.exec_time_ns
        kernel.last_profile = res

    outs = [np.asarray(r["out"], np.float64) for r in res.results]
    yuk_s = sum(o[:, 0].sum() for o in outs)
    yuk_d = sum(o[:, 1].sum() for o in outs)
    H2s = sum(o[0:64, 2] + o[64:128, 2] for o in outs)
    H2d = sum(o[0:64, 3] + o[64:128, 3] for o in outs)
    H2e = sum(o[0:64, 4] for o in outs)

    # diagonal (i==j) pairs were included in the same-spin MLP sums with
    # feats == 0; subtract their exact contribution (1024 pairs total)
    def h2_zero(b0_, W1_, b1_):
        h1 = np.tanh(b0_.astype(np.float64))
        return np.tanh(h1 @ W1_.astype(np.float64) + b1_.astype(np.float64))

    mlp_s = H2s @ W2s[:, 0].astype(np.float64) - N_EL * (
        h2_zero(b0s, W1s, b1s) @ W2s[:, 0].astype(np.float64)
    )
    mlp_d = H2d @ W2d[:, 0].astype(np.float64)

    logpsi = A_sp_s * yuk_s + A_sp_d * yuk_d + sc_s * mlp_s + sc_d * mlp_d

    emb_sum = H2e @ We2_.astype(np.float64) + N_EL * be2_.astype(np.float64)
    jastrows = emb_sum * mscale + N_EL * np.array([0.0, lbias])
    log_J = jastrows[1]
    sign = np.sign(log_J)
    logpsi = logpsi + jastrows[0] + np.log(np.abs(log_J))

    return (np.float32(sign), np.float32(logpsi))


# revision 6
# speedup vs baseline: 1.0044x; 1.0044x over previous
"""Trainium2 Bass kernel for nn_Jastrow (1024-electron pairwise Jastrow factor).

Strategy (8 NeuronCores, data-parallel over pair rows):
  - Core k owns electron rows i in [128k, 128k+128) and ALL j: 128x1024 pairs.
  - Pair features are generated on-chip in [128 i, 1024 j] layout from a
    host-broadcast copy of the electron coordinates (no 1M-row gather ever
    touches HBM).  Columns are host-permuted so j in [0,512) is always the
    same-spin half for that core's rows.
  - The two tiny pair MLPs (4->64->64, tanh) run on the PE in float32r with
    TWO pairs packed per moving column (block-diagonal weights, K=8 / K=128,
    M=128), so PE and ACT process 2 pairs per column.
  - The 64->1 output layer and all scalar postprocessing (softplus, sqrt,
    log, sign, Yukawa prefactors, diagonal-pair correction) collapse into a
    host-side fp64 epilogue on the 8 cores' partial sums:
        sum_p mlp(f_p) = W2 . sum_p h2_p
  - Yukawa cusp term expm1(-d/F)/d is computed per pair in fp32 on DVE/ACT
    and free-dim-reduced on the fly (fused (e-1)*u with accum_out).
  - Per-core device output is just [128, 8] of partial sums.
"""
import os
import sys

sys.path.insert(0, "/opt/trn_rl_repo")

import numpy as np

import concourse.bacc as bacc
import concourse.mybir as mybir
from concourse import tile
from concourse.bass_utils import run_bass_kernel_spmd

AF = mybir.ActivationFunctionType
OP = mybir.AluOpType
F32 = mybir.dt.float32
F32R = mybir.dt.float32r

N_EL = 1024
N_UP = 512
D_EMB = 256
WIDTH = 64
NC = 8
ROWS = N_EL // NC  # 128 i-rows per core
HALF = 512  # j-columns per spin half
PACK_COLS = ROWS * HALF // 2  # 32768 packed pair-columns per half (2 pairs/col)
CHUNK = 8192  # packed columns per feats tile
PSCH = 2048  # packed columns per PSUM tile (4 banks)


def _build_program(exp_scale_same, exp_scale_diff):
    nc = bacc.Bacc("TRN2", target_bir_lowering=False, debug=False)

    # ---- I/O ----
    elb_in = nc.dram_tensor("elb", [3, ROWS, N_EL], F32, kind="ExternalInput")
    eli_in = nc.dram_tensor("eli", [ROWS, 3], F32, kind="ExternalInput")
    nm_in = nc.dram_tensor("nm", [ROWS, N_EL], F32, kind="ExternalInput")
    w0s_in = nc.dram_tensor("w0s", [128, 128], F32R, kind="ExternalInput")
    w1s_in = nc.dram_tensor("w1s", [128, 128], F32R, kind="ExternalInput")
    w0d_in = nc.dram_tensor("w0d", [128, 128], F32R, kind="ExternalInput")
    w1d_in = nc.dram_tensor("w1d", [128, 128], F32R, kind="ExternalInput")
    b0s_in = nc.dram_tensor("b0s", [128, 1], F32, kind="ExternalInput")
    b1s_in = nc.dram_tensor("b1s", [128, 1], F32, kind="ExternalInput")
    b0d_in = nc.dram_tensor("b0d", [128, 1], F32, kind="ExternalInput")
    b1d_in = nc.dram_tensor("b1d", [128, 1], F32, kind="ExternalInput")
    embt_in = nc.dram_tensor("embt", [2, 128, ROWS], F32, kind="ExternalInput")
    we0_in = nc.dram_tensor("we0", [2, 128, WIDTH], F32, kind="ExternalInput")
    we1_in = nc.dram_tensor("we1", [WIDTH, WIDTH], F32, kind="ExternalInput")
    be0_in = nc.dram_tensor("be0", [WIDTH, 1], F32, kind="ExternalInput")
    be1_in = nc.dram_tensor("be1", [WIDTH, 1], F32, kind="ExternalInput")
    out_dram = nc.dram_tensor("out", [128, 8], F32, kind="ExternalOutput")

    with tile.TileContext(nc) as tc:
        with (
            tc.tile_pool(name="cst", bufs=1) as cst,
            tc.tile_pool(name="wrk", bufs=2) as wrk,
            tc.tile_pool(name="psum", bufs=1, space="PSUM") as psum,
        ):
            # ---- load persistent tiles ----
            elbx = cst.tile([ROWS, N_EL], F32, tag="elbx")
            elby = cst.tile([ROWS, N_EL], F32, tag="elby")
            elbz = cst.tile([ROWS, N_EL], F32, tag="elbz")
            nc.sync.dma_start(elbx[:], elb_in[0])
            nc.sync.dma_start(elby[:], elb_in[1])
            nc.sync.dma_start(elbz[:], elb_in[2])
            eli = cst.tile([ROWS, 3], F32, tag="eli")
            nc.sync.dma_start(eli[:], eli_in[:])
            nm = cst.tile([ROWS, N_EL], F32, tag="nm")
            nc.sync.dma_start(nm[:], nm_in[:])

            w0 = {}
            w1 = {}
            b0 = {}
            b1 = {}
            for h, (w0_in_, w1_in_, b0_in_, b1_in_) in enumerate(
                [(w0s_in, w1s_in, b0s_in, b1s_in), (w0d_in, w1d_in, b0d_in, b1d_in)]
            ):
                w0[h] = cst.tile([128, 128], F32R, tag=f"w0_{h}", name=f"w0_{h}")
                nc.sync.dma_start(w0[h][:], w0_in_[:])
                w1[h] = cst.tile([128, 128], F32R, tag=f"w1_{h}", name=f"w1_{h}")
                nc.sync.dma_start(w1[h][:], w1_in_[:])
                b0[h] = cst.tile([128, 1], F32, tag=f"b0_{h}", name=f"b0_{h}")
                nc.sync.dma_start(b0[h][:], b0_in_[:])
                b1[h] = cst.tile([128, 1], F32, tag=f"b1_{h}", name=f"b1_{h}")
                nc.sync.dma_start(b1[h][:], b1_in_[:])

            embt = cst.tile([128, 2, ROWS], F32, tag="embt")
            nc.sync.dma_start(embt[:, 0, :], embt_in[0])
            nc.sync.dma_start(embt[:, 1, :], embt_in[1])
            we0 = cst.tile([128, 2, WIDTH], F32, tag="we0")
            nc.sync.dma_start(we0[:, 0, :], we0_in[0])
            nc.sync.dma_start(we0[:, 1, :], we0_in[1])
            we1 = cst.tile([WIDTH, WIDTH], F32, tag="we1")
            nc.sync.dma_start(we1[:], we1_in[:])
            be0 = cst.tile([WIDTH, 1], F32, tag="be0")
            nc.sync.dma_start(be0[:], be0_in[:])
            be1 = cst.tile([WIDTH, 1], F32, tag="be1")
            nc.sync.dma_start(be1[:], be1_in[:])

            # ---- pair features, [128 i, 1024 j] planes ----
            # d{x,y,z} = el_i - el_j  (one fused op each)
            dx = cst.tile([ROWS, N_EL], F32, tag="dx")
            dy = cst.tile([ROWS, N_EL], F32, tag="dy")
            dz = cst.tile([ROWS, N_EL], F32, tag="dz")
            nc.vector.tensor_scalar(dx[:], elbx[:], -1.0, eli[:, 0:1], OP.mult, OP.add)
            nc.vector.tensor_scalar(dy[:], elby[:], -1.0, eli[:, 1:2], OP.mult, OP.add)
            nc.vector.tensor_scalar(dz[:], elbz[:], -1.0, eli[:, 2:3], OP.mult, OP.add)

            sq0 = cst.tile([ROWS, N_EL], F32, tag="sq0")
            sq1 = cst.tile([ROWS, N_EL], F32, tag="sq1")
            r2 = cst.tile([ROWS, N_EL], F32, tag="r2")
            nc.vector.tensor_tensor(sq0[:], dx[:], dx[:], OP.mult)
            nc.vector.tensor_tensor(sq1[:], dy[:], dy[:], OP.mult)
            nc.vector.tensor_tensor(sq0[:], sq0[:], sq1[:], OP.add)
            nc.vector.tensor_tensor(sq1[:], dz[:], dz[:], OP.mult)
            nc.vector.tensor_tensor(r2[:], sq0[:], sq1[:], OP.add)

            s = cst.tile([ROWS, N_EL], F32, tag="s")  # r = |diff|
            nc.scalar.activation(s[:], r2[:], AF.Sqrt)
            rs = cst.tile([ROWS, N_EL], F32, tag="rs")  # r + [i==j]
            nc.vector.tensor_tensor(rs[:], s[:], nm[:], OP.add)
            u = cst.tile([ROWS, N_EL], F32, tag="u")  # 1/rs
            nc.vector.reciprocal(u[:], rs[:])
            t = cst.tile([ROWS, N_EL], F32, tag="t")  # log1p(r)
            nc.scalar.activation(t[:], s[:], AF.Ln, bias=1.0)
            w = cst.tile([ROWS, N_EL], F32, tag="w")  # log1p(r)/rs
            nc.vector.tensor_tensor(w[:], t[:], u[:], OP.mult)

            # feats ping-pong buffers: rows 0-7 hold packed features, rows
            # 8-127 stay zero so L1 can run as K=128 (same PE tiling mode as
            # L2 -- avoids the ~1.2us PE mode-switch drain per matmul group)
            f8ab = []
            for nm_ in ("f8a", "f8b"):
                fb = cst.tile([128, CHUNK], F32R, tag=nm_, name=nm_)
                nc.gpsimd.memset(fb[:].bitcast(mybir.dt.uint32), 0)
                f8ab.append(fb)

            # f32r feature planes for the MLP
            dxw = cst.tile([ROWS, N_EL], F32R, tag="dxw")
            dyw = cst.tile([ROWS, N_EL], F32R, tag="dyw")
            dzw = cst.tile([ROWS, N_EL], F32R, tag="dzw")
            tfr = cst.tile([ROWS, N_EL], F32R, tag="tfr")
            nc.vector.tensor_tensor(dxw[:], dx[:], w[:], OP.mult)
            nc.vector.tensor_tensor(dyw[:], dy[:], w[:], OP.mult)
            nc.vector.tensor_tensor(dzw[:], dz[:], w[:], OP.mult)
            nc.vector.tensor_copy(tfr[:], t[:])

            # ---- Yukawa cusp: sum over pairs of expm1(-r/F)/r, per half ----
            yukred = {}
            for h, esc in enumerate([exp_scale_same, exp_scale_diff]):
                cols = slice(h * HALF, (h + 1) * HALF)
                e = wrk.tile([ROWS, HALF], F32, tag="e")
                nc.scalar.activation(e[:], s[:, cols], AF.Exp, scale=float(esc))
                ydump = wrk.tile([ROWS, HALF], F32, tag="ydump")
                yukred[h] = cst.tile([ROWS, 1], F32, tag=f"yukred{h}", name=f"yukred{h}")
                nc.vector.scalar_tensor_tensor(
                    ydump[:], e[:], 1.0, u[:, cols], OP.subtract, OP.mult,
                    accum_out=yukred[h][:],
                )

            # ---- pair MLPs: pack 2 pairs per column, f32r matmuls ----
            planes = (dxw, dyw, dzw, tfr)
            accred = {}
            for h in (0, 1):
                acc = cst.tile([128, PACK_COLS // PSCH], F32, tag=f"acc{h}")
                for c in range(PACK_COLS // CHUNK):
                    f8 = f8ab[c % 2]
                    rows_per = CHUNK // HALF  # 16 i-rows per chunk per group
                    for g in (0, 1):
                        r0 = 64 * g + rows_per * c
                        for pl, plane in enumerate(planes):
                            p = 4 * g + pl
                            nc.sync.dma_start(
                                f8[p : p + 1, :],
                                plane[r0 : r0 + rows_per, h * HALF : (h + 1) * HALF],
                            )
                    for q in range(CHUNK // PSCH):
                        ps_a = psum.tile([128, PSCH], F32, tag="A")
                        for r in range(PSCH // 512):
                            c0 = PSCH * q + 512 * r
                            nc.tensor.matmul(
                                ps_a[:, 512 * r : 512 * (r + 1)],
                                w0[h][:],
                                f8[0:128, c0 : c0 + 512],
                                start=True,
                                stop=True,
                            )
                        h1 = wrk.tile([128, PSCH], F32R, tag="h1")
                        nc.scalar.activation(h1[:], ps_a[:], AF.Tanh, bias=b0[h][:])
                        ps_b = psum.tile([128, PSCH], F32, tag="B")
                        for r in range(PSCH // 512):
                            nc.tensor.matmul(
                                ps_b[:, 512 * r : 512 * (r + 1)],
                                w1[h][:],
                                h1[:, 512 * r : 512 * (r + 1)],
                                start=True,
                                stop=True,
                            )
                        scrap = wrk.tile([128, PSCH], F32, tag="scrap")
                        idx = c * (CHUNK // PSCH) + q
                        nc.scalar.activation(
                            scrap[:], ps_b[:], AF.Tanh, bias=b1[h][:],
                            accum_out=acc[:, idx : idx + 1],
                        )
                accred[h] = cst.tile([128, 1], F32, tag=f"accred{h}", name=f"accred{h}")
                nc.vector.tensor_reduce(accred[h][:], acc[:], mybir.AxisListType.X, OP.add)

            # ---- per-electron embedding MLP (rows i0..i0+127 of embeddings) ----
            ps_e = psum.tile([WIDTH, ROWS], F32, tag="A")
            nc.tensor.matmul(ps_e[:], we0[:, 0, :], embt[:, 0, :], start=True, stop=False)
            nc.tensor.matmul(ps_e[:], we0[:, 1, :], embt[:, 1, :], start=False, stop=True)
            h1e = cst.tile([WIDTH, ROWS], F32, tag="h1e")
            nc.scalar.activation(h1e[:], ps_e[:], AF.Tanh, bias=be0[:])
            ps_e2 = psum.tile([WIDTH, ROWS], F32, tag="B")
            nc.tensor.matmul(ps_e2[:], we1[:], h1e[:], start=True, stop=True)
            h2e = cst.tile([WIDTH, ROWS], F32, tag="h2e")
            h2eacc = cst.tile([WIDTH, 1], F32, tag="h2eacc")
            nc.scalar.activation(
                h2e[:], ps_e2[:], AF.Tanh, bias=be1[:], accum_out=h2eacc[:]
            )

            # ---- outputs ----
            nc.sync.dma_start(out_dram[:, 0:1], yukred[0][:])
            nc.sync.dma_start(out_dram[:, 1:2], yukred[1][:])
            nc.sync.dma_start(out_dram[:, 2:3], accred[0][:])
            nc.sync.dma_start(out_dram[:, 3:4], accred[1][:])
            nc.sync.dma_start(out_dram[0:WIDTH, 4:5], h2eacc[:])

    nc.compile()
    return nc


_CACHE = {}


def _softplus(x):
    x = np.float64(x)
    return np.logaddexp(0.0, x)


def kernel(
    electrons, embeddings, A_same, A_diff,
    Ws0_same, bs0_same, Ws1_same, bs1_same, Ws2_same,
    Ws0_diff, bs0_diff, Ws1_diff, bs1_diff, Ws2_diff,
    scale_same, scale_diff,
    We0, be0, We1, be1, We2, be2, mlp_scale, log_bias,
):
    el = np.asarray(electrons, np.float32)
    emb = np.asarray(embeddings, np.float32)
    A_s64 = float(np.asarray(A_same, np.float64))
    A_d64 = float(np.asarray(A_diff, np.float64))
    W0s = np.asarray(Ws0_same, np.float32)
    W1s = np.asarray(Ws1_same, np.float32)
    W2s = np.asarray(Ws2_same, np.float32)
    b0s = np.asarray(bs0_same, np.float32)
    b1s = np.asarray(bs1_same, np.float32)
    W0d = np.asarray(Ws0_diff, np.float32)
    W1d = np.asarray(Ws1_diff, np.float32)
    W2d = np.asarray(Ws2_diff, np.float32)
    b0d = np.asarray(bs0_diff, np.float32)
    b1d = np.asarray(bs1_diff, np.float32)
    We0_ = np.asarray(We0, np.float32)
    We1_ = np.asarray(We1, np.float32)
    We2_ = np.asarray(We2, np.float32)
    be0_ = np.asarray(be0, np.float32)
    be1_ = np.asarray(be1, np.float32)
    be2_ = np.asarray(be2, np.float32)
    mscale = np.asarray(mlp_scale, np.float64)
    lbias = float(np.asarray(log_bias, np.float64))
    sc_s = float(np.asarray(scale_same, np.float64))
    sc_d = float(np.asarray(scale_diff, np.float64))

    A_sp_s = _softplus(A_s64)
    A_sp_d = _softplus(A_d64)
    F_s = np.sqrt(2.0 * A_sp_s)
    F_d = np.sqrt(2.0 * A_sp_d)

    key = (round(-1.0 / F_s, 12), round(-1.0 / F_d, 12))
    if key not in _CACHE:
        _CACHE[key] = _build_program(-1.0 / F_s, -1.0 / F_d)
    nc = _CACHE[key]

    # ---- block-diagonal packed weights (2 pair-groups per column) ----
    def blk(W0_, W1_, b0_, b1_):
        w0b = np.zeros((128, 128), np.float32)
        w0b[0:4, 0:64] = W0_
        w0b[4:8, 64:128] = W0_
        w1b = np.zeros((128, 128), np.float32)
        w1b[0:64, 0:64] = W1_
        w1b[64:128, 64:128] = W1_
        b0b = np.concatenate([b0_, b0_]).reshape(128, 1)
        b1b = np.concatenate([b1_, b1_]).reshape(128, 1)
        return w0b, w1b, b0b, b1b

    w0bs, w1bs, b0bs, b1bs = blk(W0s, W1s, b0s, b1s)
    w0bd, w1bd, b0bd, b1bd = blk(W0d, W1d, b0d, b1d)

    embT = emb.T.copy()  # [256, 1024]
    we0v = np.ascontiguousarray(We0_.reshape(2, 128, WIDTH))

    in_maps = []
    for k in range(NC):
        i0 = ROWS * k
        if i0 < N_UP:
            perm = np.arange(N_EL)
        else:
            perm = np.concatenate([np.arange(N_UP, N_EL), np.arange(0, N_UP)])
        elp = el[perm]  # [1024, 3] permuted so own-spin js come first
        elb = np.ascontiguousarray(
            np.broadcast_to(elp.T[:, None, :], (3, ROWS, N_EL)), np.float32
        )
        nmv = np.zeros((ROWS, N_EL), np.float32)
        rows = np.arange(ROWS)
        # global j == i0+p sits at permuted position (i0+p) % 512 in the
        # own-spin half (always columns [0, 512))
        nmv[rows, (i0 + rows) % N_UP] = 1.0
        in_maps.append(
            dict(
                elb=elb,
                eli=np.ascontiguousarray(el[i0 : i0 + ROWS]),
                nm=nmv,
                w0s=w0bs, w1s=w1bs, w0d=w0bd, w1d=w1bd,
                b0s=b0bs, b1s=b1bs, b0d=b0bd, b1d=b1bd,
                embt=np.ascontiguousarray(
                    embT[:, i0 : i0 + ROWS].reshape(2, 128, ROWS)
                ),
                we0=we0v,
                we1=We1_,
                be0=be0_.reshape(WIDTH, 1),
                be1=be1_.reshape(WIDTH, 1),
            )
        )

    trace = bool(int(os.environ.get("KERNEL_TRACE", "0")))
    res = run_bass_kernel_spmd(nc, in_maps, list(range(NC)), trace=trace)
    if trace:
        print(f"HW exec time: {res.exec_time_ns} ns")
        kernel.last_exec_time_ns = res.exec_time_ns
        kernel.last_profile = res

    outs = [np.asarray(r["out"], np.float64) for r in res.results]
    yuk_s = sum(o[:, 0].sum() for o in outs)
    yuk_d = sum(o[:, 1].sum() for o in outs)
    H2s = sum(o[0:64, 2] + o[64:128, 2] for o in outs)
    H2d = sum(o[0:64, 3] + o[64:128, 3] for o in outs)
    H2e = sum(o[0:64, 4] for o in outs)

    # diagonal (i==j) pairs were included in the same-spin MLP sums with
    # feats == 0; subtract their exact contribution (1024 pairs total)
    def h2_zero(b0_, W1_, b1_):
        h1 = np.tanh(b0_.astype(np.float64))
        return np.tanh(h1 @ W1_.astype(np.float64) + b1_.astype(np.float64))

    mlp_s = H2s @ W2s[:, 0].astype(np.float64) - N_EL * (
        h2_zero(b0s, W1s, b1s) @ W2s[:, 0].astype(np.float64)
    )
    mlp_d = H2d @ W2d[:, 0].astype(np.float64)

    logpsi = A_sp_s * yuk_s + A_sp_d * yuk_d + sc_s * mlp_s + sc_d * mlp_d

    emb_sum = H2e @ We2_.astype(np.float64) + N_EL * be2_.astype(np.float64)
    jastrows = emb_sum * mscale + N_EL * np.array([0.0, lbias])
    log_J = jastrows[1]
    sign = np.sign(log_J)
    logpsi = logpsi + jastrows[0] + np.log(np.abs(log_J))

    return (np.float32(sign), np.float32(logpsi))


# revision 8
# speedup vs baseline: 1.3459x; 1.3400x over previous
"""Trainium2 Bass kernel for nn_Jastrow (1024-electron pairwise Jastrow factor).

Strategy (8 NeuronCores, data-parallel over pair rows):
  - Core k owns electron rows i in [128k, 128k+128) and ALL j: 128x1024 pairs.
  - Pair features are generated on-chip in [128 i, 1024 j] layout from a
    host-broadcast copy of the electron coordinates (no 1M-row gather ever
    touches HBM).  Columns are host-permuted so j in [0,512) is always the
    same-spin half for that core's rows.
  - The two tiny pair MLPs (4->64->64, tanh) run on the PE in float32r with
    TWO pairs packed per moving column (block-diagonal weights, K=8 / K=128,
    M=128), so PE and ACT process 2 pairs per column.
  - The 64->1 output layer and all scalar postprocessing (softplus, sqrt,
    log, sign, Yukawa prefactors, diagonal-pair correction) collapse into a
    host-side fp64 epilogue on the 8 cores' partial sums:
        sum_p mlp(f_p) = W2 . sum_p h2_p
  - Yukawa cusp term expm1(-d/F)/d is computed per pair in fp32 on DVE/ACT
    and free-dim-reduced on the fly (fused (e-1)*u with accum_out).
  - Per-core device output is just [128, 8] of partial sums.
"""
import os
import sys

sys.path.insert(0, "/opt/trn_rl_repo")

import numpy as np

import concourse.bacc as bacc
import concourse.mybir as mybir
from concourse import tile
from concourse.bass_utils import run_bass_kernel_spmd

AF = mybir.ActivationFunctionType
OP = mybir.AluOpType
F32 = mybir.dt.float32
F32R = mybir.dt.float32r

N_EL = 1024
N_UP = 512
D_EMB = 256
WIDTH = 64
NC = 8
ROWS = N_EL // NC  # 128 i-rows per core
HALF = 512  # j-columns per spin half
PACK_COLS = ROWS * HALF // 2  # 32768 packed pair-columns per half (2 pairs/col)
CHUNK = 8192  # packed columns per feats tile
PSCH = 2048  # packed columns per PSUM tile (4 banks)


def _build_program(exp_scale_same, exp_scale_diff):
    nc = bacc.Bacc("TRN2", target_bir_lowering=False, debug=False)

    # ---- I/O ----
    elb_in = nc.dram_tensor("elb", [3, ROWS, N_EL], F32, kind="ExternalInput")
    eli_in = nc.dram_tensor("eli", [ROWS, 3], F32, kind="ExternalInput")
    nm_in = nc.dram_tensor("nm", [ROWS, N_EL], F32, kind="ExternalInput")
    w0s_in = nc.dram_tensor("w0s", [128, 128], F32R, kind="ExternalInput")
    w1s_in = nc.dram_tensor("w1s", [128, 128], F32R, kind="ExternalInput")
    w0d_in = nc.dram_tensor("w0d", [128, 128], F32R, kind="ExternalInput")
    w1d_in = nc.dram_tensor("w1d", [128, 128], F32R, kind="ExternalInput")
    b0s_in = nc.dram_tensor("b0s", [128, 1], F32, kind="ExternalInput")
    b1s_in = nc.dram_tensor("b1s", [128, 1], F32, kind="ExternalInput")
    b0d_in = nc.dram_tensor("b0d", [128, 1], F32, kind="ExternalInput")
    b1d_in = nc.dram_tensor("b1d", [128, 1], F32, kind="ExternalInput")
    embt_in = nc.dram_tensor("embt", [2, 128, ROWS], F32, kind="ExternalInput")
    we0_in = nc.dram_tensor("we0", [2, 128, WIDTH], F32, kind="ExternalInput")
    we1_in = nc.dram_tensor("we1", [WIDTH, WIDTH], F32, kind="ExternalInput")
    be0_in = nc.dram_tensor("be0", [WIDTH, 1], F32, kind="ExternalInput")
    be1_in = nc.dram_tensor("be1", [WIDTH, 1], F32, kind="ExternalInput")
    out_dram = nc.dram_tensor("out", [128, 8], F32, kind="ExternalOutput")

    with tile.TileContext(nc) as tc:
        with (
            tc.tile_pool(name="cst", bufs=1) as cst,
            tc.tile_pool(name="wrk", bufs=2) as wrk,
            tc.tile_pool(name="psum", bufs=2, space="PSUM") as psum,
        ):
            # ---- load persistent tiles ----
            elbx = cst.tile([ROWS, N_EL], F32, tag="elbx")
            elby = cst.tile([ROWS, N_EL], F32, tag="elby")
            elbz = cst.tile([ROWS, N_EL], F32, tag="elbz")
            nc.sync.dma_start(elbx[:], elb_in[0])
            nc.sync.dma_start(elby[:], elb_in[1])
            nc.sync.dma_start(elbz[:], elb_in[2])
            eli = cst.tile([ROWS, 3], F32, tag="eli")
            nc.sync.dma_start(eli[:], eli_in[:])
            nm = cst.tile([ROWS, N_EL], F32, tag="nm")
            nc.sync.dma_start(nm[:], nm_in[:])

            w0 = {}
            w1 = {}
            b0 = {}
            b1 = {}
            for h, (w0_in_, w1_in_, b0_in_, b1_in_) in enumerate(
                [(w0s_in, w1s_in, b0s_in, b1s_in), (w0d_in, w1d_in, b0d_in, b1d_in)]
            ):
                w0[h] = cst.tile([128, 128], F32R, tag=f"w0_{h}", name=f"w0_{h}")
                nc.sync.dma_start(w0[h][:], w0_in_[:])
                w1[h] = cst.tile([128, 128], F32R, tag=f"w1_{h}", name=f"w1_{h}")
                nc.sync.dma_start(w1[h][:], w1_in_[:])
                b0[h] = cst.tile([128, 1], F32, tag=f"b0_{h}", name=f"b0_{h}")
                nc.sync.dma_start(b0[h][:], b0_in_[:])
                b1[h] = cst.tile([128, 1], F32, tag=f"b1_{h}", name=f"b1_{h}")
                nc.sync.dma_start(b1[h][:], b1_in_[:])

            embt = cst.tile([128, 2, ROWS], F32, tag="embt")
            nc.sync.dma_start(embt[:, 0, :], embt_in[0])
            nc.sync.dma_start(embt[:, 1, :], embt_in[1])
            we0 = cst.tile([128, 2, WIDTH], F32, tag="we0")
            nc.sync.dma_start(we0[:, 0, :], we0_in[0])
            nc.sync.dma_start(we0[:, 1, :], we0_in[1])
            we1 = cst.tile([WIDTH, WIDTH], F32, tag="we1")
            nc.sync.dma_start(we1[:], we1_in[:])
            be0 = cst.tile([WIDTH, 1], F32, tag="be0")
            nc.sync.dma_start(be0[:], be0_in[:])
            be1 = cst.tile([WIDTH, 1], F32, tag="be1")
            nc.sync.dma_start(be1[:], be1_in[:])

            # ---- pair features, [128 i, 1024 j] planes ----
            # d{x,y,z} = el_i - el_j  (one fused op each)
            dx = cst.tile([ROWS, N_EL], F32, tag="dx")
            dy = cst.tile([ROWS, N_EL], F32, tag="dy")
            dz = cst.tile([ROWS, N_EL], F32, tag="dz")
            nc.vector.tensor_scalar(dx[:], elbx[:], -1.0, eli[:, 0:1], OP.mult, OP.add)
            nc.vector.tensor_scalar(dy[:], elby[:], -1.0, eli[:, 1:2], OP.mult, OP.add)
            nc.vector.tensor_scalar(dz[:], elbz[:], -1.0, eli[:, 2:3], OP.mult, OP.add)

            sq0 = cst.tile([ROWS, N_EL], F32, tag="sq0")
            sq1 = cst.tile([ROWS, N_EL], F32, tag="sq1")
            r2 = cst.tile([ROWS, N_EL], F32, tag="r2")
            nc.vector.tensor_tensor(sq0[:], dx[:], dx[:], OP.mult)
            nc.vector.tensor_tensor(sq1[:], dy[:], dy[:], OP.mult)
            nc.vector.tensor_tensor(sq0[:], sq0[:], sq1[:], OP.add)
            nc.vector.tensor_tensor(sq1[:], dz[:], dz[:], OP.mult)
            nc.vector.tensor_tensor(r2[:], sq0[:], sq1[:], OP.add)

            s = cst.tile([ROWS, N_EL], F32, tag="s")  # r = |diff|
            nc.scalar.activation(s[:], r2[:], AF.Sqrt)
            rs = cst.tile([ROWS, N_EL], F32, tag="rs")  # r + [i==j]
            nc.vector.tensor_tensor(rs[:], s[:], nm[:], OP.add)
            u = cst.tile([ROWS, N_EL], F32, tag="u")  # 1/rs
            nc.vector.reciprocal(u[:], rs[:])
            t = cst.tile([ROWS, N_EL], F32, tag="t")  # log1p(r)
            nc.scalar.activation(t[:], s[:], AF.Ln, bias=1.0)
            w = cst.tile([ROWS, N_EL], F32, tag="w")  # log1p(r)/rs
            nc.vector.tensor_tensor(w[:], t[:], u[:], OP.mult)

            # feats ping-pong buffers: rows 0-7 hold packed features, rows
            # 8-127 stay zero so L1 can run as K=128 (same PE tiling mode as
            # L2 -- avoids the ~1.2us PE mode-switch drain per matmul group)
            f8ab = []
            for nm_ in ("f8a", "f8b"):
                fb = cst.tile([128, CHUNK], F32R, tag=nm_, name=nm_)
                nc.gpsimd.memset(fb[:].bitcast(mybir.dt.uint32), 0)
                f8ab.append(fb)

            # f32r feature planes for the MLP
            dxw = cst.tile([ROWS, N_EL], F32R, tag="dxw")
            dyw = cst.tile([ROWS, N_EL], F32R, tag="dyw")
            dzw = cst.tile([ROWS, N_EL], F32R, tag="dzw")
            tfr = cst.tile([ROWS, N_EL], F32R, tag="tfr")
            nc.vector.tensor_tensor(dxw[:], dx[:], w[:], OP.mult)
            nc.vector.tensor_tensor(dyw[:], dy[:], w[:], OP.mult)
            nc.vector.tensor_tensor(dzw[:], dz[:], w[:], OP.mult)
            nc.vector.tensor_copy(tfr[:], t[:])

            # ---- Yukawa cusp: sum over pairs of expm1(-r/F)/r, per half ----
            yukred = {}
            for h, esc in enumerate([exp_scale_same, exp_scale_diff]):
                cols = slice(h * HALF, (h + 1) * HALF)
                e = wrk.tile([ROWS, HALF], F32, tag="e")
                nc.scalar.activation(e[:], s[:, cols], AF.Exp, scale=float(esc))
                ydump = wrk.tile([ROWS, HALF], F32, tag="ydump")
                yukred[h] = cst.tile([ROWS, 1], F32, tag=f"yukred{h}", name=f"yukred{h}")
                nc.vector.scalar_tensor_tensor(
                    ydump[:], e[:], 1.0, u[:, cols], OP.subtract, OP.mult,
                    accum_out=yukred[h][:],
                )

            # ---- pair MLPs: pack 2 pairs per column, f32r matmuls ----
            planes = (dxw, dyw, dzw, tfr)
            accred = {}
            for h in (0, 1):
                acc = cst.tile([128, PACK_COLS // PSCH], F32, tag=f"acc{h}")
                for c in range(PACK_COLS // CHUNK):
                    f8 = f8ab[c % 2]
                    rows_per = CHUNK // HALF  # 16 i-rows per chunk per group
                    for g in (0, 1):
                        r0 = 64 * g + rows_per * c
                        for pl, plane in enumerate(planes):
                            p = 4 * g + pl
                            nc.sync.dma_start(
                                f8[p : p + 1, :],
                                plane[r0 : r0 + rows_per, h * HALF : (h + 1) * HALF],
                            )
                    for q in range(CHUNK // PSCH):
                        # one psum tile serves BOTH layers: mm1 fills it,
                        # tanhA drains it to SBUF, mm2 overwrites it, tanhB
                        # drains again.  With bufs=2 two chunk-chains are in
                        # flight, so ACT never waits on the mm2 latency.
                        ps_a = psum.tile([128, PSCH], F32, tag="A")
                        for r in range(PSCH // 512):
                            c0 = PSCH * q + 512 * r
                            nc.tensor.matmul(
                                ps_a[:, 512 * r : 512 * (r + 1)],
                                w0[h][:],
                                f8[0:128, c0 : c0 + 512],
                                start=True,
                                stop=True,
                            )
                        h1 = wrk.tile([128, PSCH], F32R, tag="h1")
                        nc.scalar.activation(h1[:], ps_a[:], AF.Tanh, bias=b0[h][:])
                        for r in range(PSCH // 512):
                            nc.tensor.matmul(
                                ps_a[:, 512 * r : 512 * (r + 1)],
                                w1[h][:],
                                h1[:, 512 * r : 512 * (r + 1)],
                                start=True,
                                stop=True,
                            )
                        scrap = wrk.tile([128, PSCH], F32, tag="scrap")
                        idx = c * (CHUNK // PSCH) + q
                        nc.scalar.activation(
                            scrap[:], ps_a[:], AF.Tanh, bias=b1[h][:],
                            accum_out=acc[:, idx : idx + 1],
                        )
                accred[h] = cst.tile([128, 1], F32, tag=f"accred{h}", name=f"accred{h}")
                nc.vector.tensor_reduce(accred[h][:], acc[:], mybir.AxisListType.X, OP.add)

            # ---- per-electron embedding MLP (rows i0..i0+127 of embeddings) ----
            ps_e = psum.tile([WIDTH, ROWS], F32, tag="A")
            nc.tensor.matmul(ps_e[:], we0[:, 0, :], embt[:, 0, :], start=True, stop=False)
            nc.tensor.matmul(ps_e[:], we0[:, 1, :], embt[:, 1, :], start=False, stop=True)
            h1e = cst.tile([WIDTH, ROWS], F32, tag="h1e")
            nc.scalar.activation(h1e[:], ps_e[:], AF.Tanh, bias=be0[:])
            ps_e2 = psum.tile([WIDTH, ROWS], F32, tag="A")
            nc.tensor.matmul(ps_e2[:], we1[:], h1e[:], start=True, stop=True)
            h2e = cst.tile([WIDTH, ROWS], F32, tag="h2e")
            h2eacc = cst.tile([WIDTH, 1], F32, tag="h2eacc")
            nc.scalar.activation(
                h2e[:], ps_e2[:], AF.Tanh, bias=be1[:], accum_out=h2eacc[:]
            )

            # ---- outputs ----
            nc.sync.dma_start(out_dram[:, 0:1], yukred[0][:])
            nc.sync.dma_start(out_dram[:, 1:2], yukred[1][:])
            nc.sync.dma_start(out_dram[:, 2:3], accred[0][:])
            nc.sync.dma_start(out_dram[:, 3:4], accred[1][:])
            nc.sync.dma_start(out_dram[0:WIDTH, 4:5], h2eacc[:])

    nc.compile()
    return nc


_CACHE = {}


def _softplus(x):
    x = np.float64(x)
    return np.logaddexp(0.0, x)


def kernel(
    electrons, embeddings, A_same, A_diff,
    Ws0_same, bs0_same, Ws1_same, bs1_same, Ws2_same,
    Ws0_diff, bs0_diff, Ws1_diff, bs1_diff, Ws2_diff,
    scale_same, scale_diff,
    We0, be0, We1, be1, We2, be2, mlp_scale, log_bias,
):
    el = np.asarray(electrons, np.float32)
    emb = np.asarray(embeddings, np.float32)
    A_s64 = float(np.asarray(A_same, np.float64))
    A_d64 = float(np.asarray(A_diff, np.float64))
    W0s = np.asarray(Ws0_same, np.float32)
    W1s = np.asarray(Ws1_same, np.float32)
    W2s = np.asarray(Ws2_same, np.float32)
    b0s = np.asarray(bs0_same, np.float32)
    b1s = np.asarray(bs1_same, np.float32)
    W0d = np.asarray(Ws0_diff, np.float32)
    W1d = np.asarray(Ws1_diff, np.float32)
    W2d = np.asarray(Ws2_diff, np.float32)
    b0d = np.asarray(bs0_diff, np.float32)
    b1d = np.asarray(bs1_diff, np.float32)
    We0_ = np.asarray(We0, np.float32)
    We1_ = np.asarray(We1, np.float32)
    We2_ = np.asarray(We2, np.float32)
    be0_ = np.asarray(be0, np.float32)
    be1_ = np.asarray(be1, np.float32)
    be2_ = np.asarray(be2, np.float32)
    mscale = np.asarray(mlp_scale, np.float64)
    lbias = float(np.asarray(log_bias, np.float64))
    sc_s = float(np.asarray(scale_same, np.float64))
    sc_d = float(np.asarray(scale_diff, np.float64))

    A_sp_s = _softplus(A_s64)
    A_sp_d = _softplus(A_d64)
    F_s = np.sqrt(2.0 * A_sp_s)
    F_d = np.sqrt(2.0 * A_sp_d)

    key = (round(-1.0 / F_s, 12), round(-1.0 / F_d, 12))
    if key not in _CACHE:
        _CACHE[key] = _build_program(-1.0 / F_s, -1.0 / F_d)
    nc = _CACHE[key]

    # ---- block-diagonal packed weights (2 pair-groups per column) ----
    def blk(W0_, W1_, b0_, b1_):
        w0b = np.zeros((128, 128), np.float32)
        w0b[0:4, 0:64] = W0_
        w0b[4:8, 64:128] = W0_
        w1b = np.zeros((128, 128), np.float32)
        w1b[0:64, 0:64] = W1_
        w1b[64:128, 64:128] = W1_
        b0b = np.concatenate([b0_, b0_]).reshape(128, 1)
        b1b = np.concatenate([b1_, b1_]).reshape(128, 1)
        return w0b, w1b, b0b, b1b

    w0bs, w1bs, b0bs, b1bs = blk(W0s, W1s, b0s, b1s)
    w0bd, w1bd, b0bd, b1bd = blk(W0d, W1d, b0d, b1d)

    embT = emb.T.copy()  # [256, 1024]
    we0v = np.ascontiguousarray(We0_.reshape(2, 128, WIDTH))

    in_maps = []
    for k in range(NC):
        i0 = ROWS * k
        if i0 < N_UP:
            perm = np.arange(N_EL)
        else:
            perm = np.concatenate([np.arange(N_UP, N_EL), np.arange(0, N_UP)])
        elp = el[perm]  # [1024, 3] permuted so own-spin js come first
        elb = np.ascontiguousarray(
            np.broadcast_to(elp.T[:, None, :], (3, ROWS, N_EL)), np.float32
        )
        nmv = np.zeros((ROWS, N_EL), np.float32)
        rows = np.arange(ROWS)
        # global j == i0+p sits at permuted position (i0+p) % 512 in the
        # own-spin half (always columns [0, 512))
        nmv[rows, (i0 + rows) % N_UP] = 1.0
        in_maps.append(
            dict(
                elb=elb,
                eli=np.ascontiguousarray(el[i0 : i0 + ROWS]),
                nm=nmv,
                w0s=w0bs, w1s=w1bs, w0d=w0bd, w1d=w1bd,
                b0s=b0bs, b1s=b1bs, b0d=b0bd, b1d=b1bd,
                embt=np.ascontiguousarray(
                    embT[:, i0 : i0 + ROWS].reshape(2, 128, ROWS)
                ),
                we0=we0v,
                we1=We1_,
                be0=be0_.reshape(WIDTH, 1),
                be1=be1_.reshape(WIDTH, 1),
            )
        )

    trace = bool(int(os.environ.get("KERNEL_TRACE", "0")))
    res = run_bass_kernel_spmd(nc, in_maps, list(range(NC)), trace=trace)
    if trace:
        print(f"HW exec time: {res.exec_time_ns} ns")
        kernel.last_exec_time_ns = res.exec_time_ns
        kernel.last_profile = res

    outs = [np.asarray(r["out"], np.float64) for r in res.results]
    yuk_s = sum(o[:, 0].sum() for o in outs)
    yuk_d = sum(o[:, 1].sum() for o in outs)
    H2s = sum(o[0:64, 2] + o[64:128, 2] for o in outs)
    H2d = sum(o[0:64, 3] + o[64:128, 3] for o in outs)
    H2e = sum(o[0:64, 4] for o in outs)

    # diagonal (i==j) pairs were included in the same-spin MLP sums with
    # feats == 0; subtract their exact contribution (1024 pairs total)
    def h2_zero(b0_, W1_, b1_):
        h1 = np.tanh(b0_.astype(np.float64))
        return np.tanh(h1 @ W1_.astype(np.float64) + b1_.astype(np.float64))

    mlp_s = H2s @ W2s[:, 0].astype(np.float64) - N_EL * (
        h2_zero(b0s, W1s, b1s) @ W2s[:, 0].astype(np.float64)
    )
    mlp_d = H2d @ W2d[:, 0].astype(np.float64)

    logpsi = A_sp_s * yuk_s + A_sp_d * yuk_d + sc_s * mlp_s + sc_d * mlp_d

    emb_sum = H2e @ We2_.astype(np.float64) + N_EL * be2_.astype(np.float64)
    jastrows = emb_sum * mscale + N_EL * np.array([0.0, lbias])
    log_J = jastrows[1]
    sign = np.sign(log_J)
    logpsi = logpsi + jastrows[0] + np.log(np.abs(log_J))

    return (np.float32(sign), np.float32(logpsi))
